# revision 24
# baseline (speedup 1.0000x reference)
"""Trainium2 Bass kernel for a dense transformer block (nn_Block_30262339567972).

Full inputs in, full outputs out. Sharded across 8 NeuronCores with zero
collectives: core c = 2*b + j owns two 512-token chunks of batch b
(j=0 -> chunks {0,3}, j=1 -> chunks {1,2}; the pairing balances causal
attention work). The HOST PERMUTES the token axis per core to
[own_lo, own_hi, other_a, other_b] so one uniform program serves both
chunk assignments: own tokens are always columns 0-1023, and all causal
structure lives in host-computed mask data + a fixed block skip/narrow
pattern that is identical for both variants.

Everything on device is feature-major and bf16 (fp32 PSUM accumulation);
LN gains/biases are folded into the following matmul's weights on the
host, so LayerNorm on device is stats (PE ones-matmuls) + (x-mu)*rs only.
K/V/Q stay in SBUF (no DRAM spill). Attention emission is software-
pipelined (scores[i+1] lands between scores[i] and y[i] on the PE queue)
so the PE never sees a multi-microsecond idle window (keeps the HAM
clock gate at 8/8). SBUF is tight, so phases alias dead buffers:
attnT overwrites qT group-by-group, x2/h2 carve out of the dead V tiles,
and the MLP intermediate h1 reuses kT's space.
"""

from contextlib import ExitStack

import numpy as np
import ml_dtypes

import concourse.bacc as bacc
import concourse.bass as bass
import concourse.tile as tile
from concourse import mybir
from concourse.bass_utils import run_bass_kernel_spmd
import concourse.bass_utils as _bu

# NOTE: ldw-opt stays at the default (false) — walrus rejects some of this
# kernel's Ldweights under --enable-ldw-opt=true (strided V-tile weights).

F32 = mybir.dt.float32
BF16 = mybir.dt.bfloat16
P = 128
B, T, C = 4, 2048, 1024
H, D = 16, 64
DFF = 4096
TOWN = 1024
KT = C // P            # 8 feature tiles
G = H // 2             # 8 head-pair groups
NB = T // 512          # 4 token blocks of 512
EPS = 1e-5
SCALE = D ** -0.5
NEG = -1e30
BF = ml_dtypes.bfloat16
DEBUG = False

Exp = mybir.ActivationFunctionType.Exp
Relu = mybir.ActivationFunctionType.Relu
Sqrt = mybir.ActivationFunctionType.Sqrt
ADD = mybir.AluOpType.add
MULT = mybir.AluOpType.mult

# Attention pair schedule, uniform across cores (permuted kv order).
# Each entry: (ktA, ktB, qsA, qsB, mask_idx)
#   kt: kv 128-token tile in permuted order
#   qs: first query column computed for that kt (diagonal narrowing)
#   mask_idx: row in the masks input, or None for never-masked pairs
PAIR_DEFS = {
    0: [
        (0, 1, 0, 128, 0),      # own_lo diagonal
        (2, 3, 256, 384, 1),
        (8, 9, 0, 0, 2),        # other_a: data mask (all-0 or all-NEG)
        (10, 11, 0, 0, 3),
    ],
    1: [
        (0, 1, 0, 0, None),     # own_lo always fully visible from own_hi
        (2, 3, 0, 0, None),
        (4, 5, 0, 128, 4),      # own_hi diagonal
        (6, 7, 256, 384, 5),
        (8, 9, 0, 0, None),     # other_a always fully visible
        (10, 11, 0, 0, None),
        (12, 13, 0, 0, 6),      # other_b: data mask
        (14, 15, 0, 0, 7),
    ],
}


def _ln_stats(nc, src_aps, ones_bf, eps_t, stp, sqp, rowp, bcp, tag):
    """Feature-major LayerNorm stats over a 512-token block.

    src_aps: KT APs of [P, 512] bf16. Returns (mu_b, rs_b) bf16 [P, 512]
    broadcast tiles."""
    ssum = stp.tile([1, 512], F32, tag="ssum", name=f"ssum{tag}")
    ssq = stp.tile([1, 512], F32, tag="ssq", name=f"ssq{tag}")
    for kt in range(KT):
        nc.tensor.matmul(ssum, ones_bf, src_aps[kt],
                         start=(kt == 0), stop=(kt == KT - 1))
    for kt in range(KT):
        sq = sqp.tile([P, 512], BF16, tag=f"sq{kt}", name=f"sq{tag}_{kt}",
                      bufs=1)
        nc.vector.tensor_mul(out=sq, in0=src_aps[kt], in1=src_aps[kt])
        nc.tensor.matmul(ssq, ones_bf, sq,
                         start=(kt == 0), stop=(kt == KT - 1))
    mu = rowp.tile([1, 512], F32, tag="mu", name=f"mu{tag}")
    nc.vector.tensor_scalar_mul(mu, ssum, 1.0 / C)
    ms = rowp.tile([1, 512], F32, tag="ms", name=f"ms{tag}")
    nc.vector.tensor_scalar_mul(ms, ssq, 1.0 / C)
    mu2 = rowp.tile([1, 512], F32, tag="mu2", name=f"mu2{tag}")
    nc.vector.tensor_mul(out=mu2, in0=mu, in1=mu)
    nc.vector.tensor_sub(out=ms, in0=ms, in1=mu2)
    std = rowp.tile([1, 512], F32, tag="std", name=f"std{tag}")
    nc.scalar.activation(out=std, in_=ms, func=Sqrt, bias=eps_t[0:1, 0:1])
    rs = rowp.tile([1, 512], F32, tag="rs", name=f"rs{tag}")
    nc.vector.reciprocal_approx_fast(out=rs, in_=std)
    mu_bf = rowp.tile([1, 512], BF16, tag="mubf", name=f"mubf{tag}")
    nc.vector.tensor_copy(out=mu_bf, in_=mu)
    rs_bf = rowp.tile([1, 512], BF16, tag="rsbf", name=f"rsbf{tag}")
    nc.vector.tensor_copy(out=rs_bf, in_=rs)
    mu_b = bcp.tile([P, 512], BF16, tag="mub", name=f"mub{tag}")
    nc.gpsimd.partition_broadcast(mu_b, mu_bf)
    rs_b = bcp.tile([P, 512], BF16, tag="rsb", name=f"rsb{tag}")
    nc.gpsimd.partition_broadcast(rs_b, rs_bf)
    return mu_b, rs_b


def build_nc():
    nc = bacc.Bacc()
    xT = nc.declare_dram_parameter("xT", [C, T], BF16, isOutput=False)
    masks = nc.declare_dram_parameter("masks", [9, P, 1024], BF16, isOutput=False)
    attn_w = nc.declare_dram_parameter("attn_w", [C, 3 * C], BF16, isOutput=False)
    attn_b = nc.declare_dram_parameter("attn_b", [3 * C], F32, isOutput=False)
    proj_w = nc.declare_dram_parameter("proj_w", [C, C], BF16, isOutput=False)
    proj_b = nc.declare_dram_parameter("proj_b", [C], F32, isOutput=False)
    fc1_w = nc.declare_dram_parameter("fc1_w", [C, DFF], BF16, isOutput=False)
    fc1_b = nc.declare_dram_parameter("fc1_b", [DFF], F32, isOutput=False)
    fc2_w = nc.declare_dram_parameter("fc2_w", [DFF, C], BF16, isOutput=False)
    fc2_b = nc.declare_dram_parameter("fc2_b", [C], F32, isOutput=False)
    out = nc.declare_dram_parameter("out", [C, TOWN], F32, isOutput=True)
    if DEBUG:
        dbg_k = nc.declare_dram_parameter("dbg_k", [P, T], BF16, isOutput=True)
        dbg_q = nc.declare_dram_parameter("dbg_q", [P, TOWN], BF16, isOutput=True)
        dbg_v = nc.declare_dram_parameter("dbg_v", [P, 1040], BF16, isOutput=True)
        dbg_at = nc.declare_dram_parameter("dbg_at", [P, TOWN], BF16, isOutput=True)
        dbg_x2 = nc.declare_dram_parameter("dbg_x2", [P, TOWN], BF16, isOutput=True)
        dbg_h2 = nc.declare_dram_parameter("dbg_h2", [P, TOWN], BF16, isOutput=True)
        dbg_den = nc.declare_dram_parameter("dbg_den", [8, 512], F32, isOutput=True)

    with tile.TileContext(nc) as tc, ExitStack() as top:
        const = top.enter_context(tc.tile_pool(name="const", bufs=1))
        ones_bf = const.tile([P, 1], BF16, name="ones_bf")
        nc.vector.memset(ones_bf, 1.0)
        eps_t = const.tile([P, 1], F32, name="eps_t")
        nc.vector.memset(eps_t, EPS)
        abq = const.tile([P, G], F32, name="abq")
        abk = const.tile([P, G], F32, name="abk")
        nc.gpsimd.dma_start(out=abq, in_=attn_b[0:C].rearrange("(g p) -> p g", p=P))
        nc.gpsimd.dma_start(out=abk, in_=attn_b[C:2 * C].rearrange("(g p) -> p g", p=P))
        projb = const.tile([P, KT], F32, name="projb")
        nc.gpsimd.dma_start(out=projb, in_=proj_b.rearrange("(f p) -> p f", p=P))
        fc2b = const.tile([P, KT], F32, name="fc2b")
        nc.gpsimd.dma_start(out=fc2b, in_=fc2_b.rearrange("(f p) -> p f", p=P))
        fc1b = const.tile([P, DFF // P], F32, name="fc1b")
        nc.gpsimd.dma_start(out=fc1b, in_=fc1_b.rearrange("(f p) -> p f", p=P))
        bv_bc = const.tile([P, C], F32, name="bv_bc")
        abv = attn_b[2 * C:3 * C]
        nc.gpsimd.dma_start(
            out=bv_bc,
            in_=bass.AP(tensor=abv.tensor, offset=abv.offset,
                        ap=[[0, P]] + list(abv.ap[-1:])))

        # Long-lived activation state; later phases alias into dead regions.
        kvq = top.enter_context(tc.tile_pool(name="kvq", bufs=1))
        kT_t = [kvq.tile([P, T], BF16, tag=f"k{g}", name=f"kT{g}")
                for g in range(G)]
        qT_t = [kvq.tile([P, TOWN], BF16, tag=f"q{g}", name=f"qT{g}")
                for g in range(G)]
        vx = [kvq.tile([P, 1040], BF16, tag=f"v{tt}", name=f"v{tt}")
              for tt in range(T // P)]
        v_t = [t.rearrange("p (g x d) -> p g x d", g=G, x=2, d=65) for t in vx]
        for tt in range(T // P):
            nc.vector.memset(v_t[tt][:, :, :, 64:65], 1.0)
        # aliases (regions dead by the time they are written):
        attnT = qT_t                                   # written per-(g,hh,qc)
        x2T = [vx[2 * ft][:, 0:TOWN] for ft in range(KT)]
        h2T = [vx[2 * ft + 1][:, 0:TOWN] for ft in range(KT)]
        h1 = [kT_t[m // 2][:, (m % 2) * TOWN:(m % 2 + 1) * TOWN]
              for m in range(16)]

        # hT (LN1 output) stays resident through attention: K groups 2-7
        # and the V upper half are produced as interleaved "chores" inside
        # the attention phase, keeping the PE in long busy bursts so the
        # HAM clock gate ramps back to 8/8 and stays there.
        ha = ExitStack()
        hTp = ha.enter_context(tc.tile_pool(name="hTp", bufs=1))
        hT_t = [hTp.tile([P, T], BF16, tag=f"ht{kt}", name=f"hT{kt}")
                for kt in range(KT)]
        awkp = ha.enter_context(tc.tile_pool(name="awkp", bufs=1))
        awk = [awkp.tile([P, C], BF16, tag=f"awk{kt}", name=f"awk{kt}")
               for kt in range(KT)]
        awvp = ha.enter_context(tc.tile_pool(name="awvp", bufs=1))
        awv = [awvp.tile([P, C], BF16, tag=f"awv{kt}", name=f"awv{kt}")
               for kt in range(KT)]

        # ================= Phase 1: LN1 + K(g0-1)/V(lo)/Q ==================
        with ExitStack() as c1:
            # x for blocks 0/1 first so LN can start ~immediately; weights
            # stream behind them.
            for nb in (0, 1):
                for kt in range(KT):
                    eng = nc.sync if kt < 4 else nc.scalar
                    eng.dma_start(
                        out=hT_t[kt][:, nb * 512:(nb + 1) * 512],
                        in_=xT[kt * P:(kt + 1) * P, nb * 512:(nb + 1) * 512])
            awqp = c1.enter_context(tc.tile_pool(name="awqp", bufs=1))
            awq = [awqp.tile([P, C], BF16, tag=f"awq{kt}", name=f"awq{kt}")
                   for kt in range(KT)]
            for kt in range(KT):
                nc.sync.dma_start(out=awk[kt],
                                  in_=attn_w[kt * P:(kt + 1) * P, C:2 * C])
                nc.scalar.dma_start(out=awv[kt],
                                    in_=attn_w[kt * P:(kt + 1) * P, 2 * C:3 * C])
                nc.gpsimd.dma_start(out=awq[kt],
                                    in_=attn_w[kt * P:(kt + 1) * P, 0:C])
            for nb in (2, 3):
                for kt in range(KT):
                    eng = nc.sync if kt < 4 else nc.scalar
                    eng.dma_start(
                        out=hT_t[kt][:, nb * 512:(nb + 1) * 512],
                        in_=xT[kt * P:(kt + 1) * P, nb * 512:(nb + 1) * 512])

            sqp = c1.enter_context(tc.tile_pool(name="sqp", bufs=1))
            stp = c1.enter_context(tc.tile_pool(name="stp", bufs=1, space="PSUM"))
            rowp = c1.enter_context(tc.tile_pool(name="rowp", bufs=1))
            bcp = c1.enter_context(tc.tile_pool(name="bcp", bufs=2))
            mmp = c1.enter_context(tc.tile_pool(name="mmp", bufs=2, space="PSUM"))

            for nb in range(NB):
                sl = slice(nb * 512, (nb + 1) * 512)
                xb = [hT_t[kt][:, sl] for kt in range(KT)]
                mu_b, rs_b = _ln_stats(nc, xb, ones_bf, eps_t, stp, sqp,
                                       rowp, bcp, f"a{nb}")
                # in-place apply: hT <- (x - mu) * rs
                for kt in range(KT):
                    nc.vector.tensor_sub(out=hT_t[kt][:, sl],
                                         in0=hT_t[kt][:, sl], in1=mu_b)
                    nc.vector.tensor_mul(out=hT_t[kt][:, sl],
                                         in0=hT_t[kt][:, sl], in1=rs_b)
                hT = xb

                for g in (0, 1):
                    kps = mmp.tile([P, 512], F32, tag="kq", name=f"kps{nb}_{g}")
                    for kt in range(KT):
                        nc.tensor.matmul(
                            kps, awk[kt][:, g * P:(g + 1) * P], hT[kt],
                            start=(kt == 0), stop=(kt == KT - 1))
                    nc.vector.tensor_scalar_add(
                        out=kT_t[g][:, sl], in0=kps, scalar1=abk[:, g:g + 1])
                if nb < 2:
                    for g in range(G):
                        qps = mmp.tile([P, 512], F32, tag="kq",
                                       name=f"qps{nb}_{g}")
                        for kt in range(KT):
                            nc.tensor.matmul(
                                qps, awq[kt][:, g * P:(g + 1) * P], hT[kt],
                                start=(kt == 0), stop=(kt == KT - 1))
                        nc.vector.tensor_scalar_add(
                            out=qT_t[g][:, sl], in0=qps,
                            scalar1=abq[:, g:g + 1])

                # --- V lower feature half (groups 0-3) ---
                for t4 in range(4):
                    tt = nb * 4 + t4
                    vps = mmp.tile([P, 512], F32, tag="v0", name=f"vps{tt}")
                    for kt in range(KT):
                        nc.tensor.matmul(
                            vps, hT[kt][:, t4 * P:(t4 + 1) * P],
                            awv[kt][:, 0:512],
                            start=(kt == 0), stop=(kt == KT - 1))
                    nc.vector.tensor_add(
                        out=v_t[tt][:, 0:4, :, 0:64],
                        in0=vps.rearrange("p (g x d) -> p g x d", x=2, d=64),
                        in1=bv_bc[:, 0:512].rearrange("p (g x d) -> p g x d",
                                                      x=2, d=64))

        if DEBUG:
            nc.sync.dma_start(out=dbg_k[0:P, :], in_=kT_t[0])
            nc.sync.dma_start(out=dbg_q[0:P, :], in_=qT_t[0])
            nc.sync.dma_start(out=dbg_v[0:P, :], in_=vx[0])

        # ================= Phase 2: attention ==============================
        with ExitStack() as cb:
            pwp = cb.enter_context(tc.tile_pool(name="pwp", bufs=1))
            pw = [pwp.tile([P, C], BF16, tag=f"pw{kt}", name=f"pw{kt}")
                  for kt in range(KT)]
            for kt in range(KT):
                nc.sync.dma_start(out=pw[kt], in_=proj_w[kt * P:(kt + 1) * P, :])

            with ExitStack() as c2:
                mkp = c2.enter_context(tc.tile_pool(name="mkp", bufs=1))
                mk = [mkp.tile([P, 1024], BF16, tag=f"mk{i}", name=f"mk{i}")
                      for i in range(9)]
                for i in range(9):
                    nc.sync.dma_start(out=mk[i], in_=masks[i])
                tri01 = mk[8][:, 0:128]
                # data masks are block-uniform: fold scale*mask into the exp
                # bias (per-partition column). Diagonal masks act post-exp as
                # a 0/1 triangle multiply on gpsimd (SBUF-only engine).
                mbias = []
                for i in range(8):
                    mb = mkp.tile([P, 1], F32, tag=f"mb{i}", name=f"mb{i}")
                    nc.vector.tensor_scalar_mul(mb, mk[i][:, 0:1], SCALE)
                    mbias.append(mb)
                scp = c2.enter_context(tc.tile_pool(name="scp", bufs=2,
                                                    space="PSUM"))
                yp = c2.enter_context(tc.tile_pool(name="yp", bufs=2,
                                                   space="PSUM"))
                chm = c2.enter_context(tc.tile_pool(name="chm", bufs=2,
                                                    space="PSUM"))

                # deferred K (groups 2-7) and V upper half, emitted as
                # chores between attention pairs: each chore is an 8-matmul
                # PSUM chain (a sustained PE burst for the HAM clock gate).
                def k_chore(g, nb):
                    def run():
                        sl = slice(nb * 512, (nb + 1) * 512)
                        kps = chm.tile([P, 512], F32, tag="ch",
                                       name=f"dkps{g}_{nb}")
                        for kt in range(KT):
                            nc.tensor.matmul(
                                kps, awk[kt][:, g * P:(g + 1) * P],
                                hT_t[kt][:, sl],
                                start=(kt == 0), stop=(kt == KT - 1))
                        nc.vector.tensor_scalar_add(
                            out=kT_t[g][:, sl], in0=kps,
                            scalar1=abk[:, g:g + 1])
                    return run

                def v_chore(tt):
                    def run():
                        vps = chm.tile([P, 512], F32, tag="ch",
                                       name=f"dvps{tt}")
                        for kt in range(KT):
                            nc.tensor.matmul(
                                vps, hT_t[kt][:, tt * P:(tt + 1) * P],
                                awv[kt][:, 512:1024],
                                start=(kt == 0), stop=(kt == KT - 1))
                        nc.vector.tensor_add(
                            out=v_t[tt][:, 4:8, :, 0:64],
                            in0=vps.rearrange("p (g x d) -> p g x d",
                                              x=2, d=64),
                            in1=bv_bc[:, 512:1024].rearrange(
                                "p (g x d) -> p g x d", x=2, d=64))
                    return run

                chores = []
                for g_ in (2, 3):
                    chores += [k_chore(g_, nb_) for nb_ in range(NB)]
                chores += [v_chore(tt_) for tt_ in range(T // P)]
                for g_ in (4, 5, 6, 7):
                    chores += [k_chore(g_, nb_) for nb_ in range(NB)]
                chores.reverse()   # pop() from the front
                ptp = c2.enter_context(tc.tile_pool(name="ptp", bufs=3))
                rcp = c2.enter_context(tc.tile_pool(name="rcp", bufs=2))
                rbp = c2.enter_context(tc.tile_pool(name="rbp", bufs=2))

                # software-pipelined emission: scores(i+1) lands on the PE
                # queue between scores(i) and y(i) so the PE never waits a
                # full mask+exp latency. Normalize is emitted right after a
                # y-group's last matmul; the pipeline flows across qc/hh/g.
                pend = [None]

                def norm(y_t, g, hh, qc):
                    hsl = slice(64 * hh, 64 * (hh + 1))
                    den = rcp.tile([1, 512], F32, tag="den",
                                   name=f"den{g}_{hh}_{qc}")
                    nc.vector.tensor_copy(out=den, in_=y_t[64:65, :])
                    rc = rcp.tile([1, 512], F32, tag="rc",
                                  name=f"rc{g}_{hh}_{qc}")
                    nc.vector.reciprocal_approx_fast(out=rc, in_=den)
                    if DEBUG and g == 0 and hh == 0:
                        nc.sync.dma_start(out=dbg_den[2 * qc:2 * qc + 1, :],
                                          in_=den)
                        nc.sync.dma_start(out=dbg_den[2 * qc + 1:2 * qc + 2, :],
                                          in_=rc)
                    rb = rbp.tile([64, 512], F32, tag="rb",
                                  name=f"rb{g}_{hh}_{qc}")
                    nc.gpsimd.partition_broadcast(rb, rc)
                    nc.vector.tensor_mul(
                        out=attnT[g][hsl, qc * 512:(qc + 1) * 512],
                        in0=y_t[0:64, :], in1=rb)

                def flush_y():
                    if pend[0] is None:
                        return
                    pts, items, y_t, first, last, g_, hh_, qc_ = pend[0]
                    for idx, (kt, off, qs, ap) in enumerate(items):
                        nc.tensor.matmul(
                            y_t[:, qs:qs + ap], v_t[kt][:, g_, hh_, :],
                            pts[:, off:off + ap],
                            start=(first and idx == 0),
                            stop=(last and idx == len(items) - 1))
                    if last:
                        norm(y_t, g_, hh_, qc_)
                    pend[0] = None

                for g in range(G):
                    for hh in range(2):
                        hsl = slice(64 * hh, 64 * (hh + 1))
                        for qc in (0, 1):
                            pairs = PAIR_DEFS[qc]
                            y_t = yp.tile([65, 512], F32, tag="y",
                                          name=f"y{g}_{hh}_{qc}")
                            for pi, (ktA, ktB, qsA, qsB, mi) in enumerate(pairs):
                                items = []
                                off = 0
                                for kt, qs in ((ktA, qsA), (ktB, qsB)):
                                    items.append((kt, off, qs, 512 - qs))
                                    off += 512 - qs
                                w = off
                                scs = scp.tile([P, 1024], F32, tag="sc",
                                               name=f"sc{g}_{hh}_{qc}_{pi}")
                                for (kt, o_, qs, ap) in items:
                                    nc.tensor.matmul(
                                        scs[:, o_:o_ + ap],
                                        kT_t[g][hsl, kt * P:(kt + 1) * P],
                                        qT_t[g][hsl,
                                                qc * 512 + qs:(qc + 1) * 512],
                                        start=True, stop=True,
                                        tile_position=(64 * hh, 0))
                                flush_y()
                                if chores:
                                    chores.pop()()
                                if chores:
                                    chores.pop()()
                                diag = mi is not None and (qsA or qsB)
                                pts = ptp.tile([P, 1024], BF16, tag="pt",
                                               name=f"pt{g}_{hh}_{qc}_{pi}")
                                nc.scalar.activation(
                                    out=pts[:, 0:w], in_=scs[:, 0:w],
                                    func=Exp, scale=SCALE,
                                    bias=(mbias[mi][:, 0:1]
                                          if (mi is not None and not diag)
                                          else 0.0))
                                if diag:
                                    for (kt, o_, qs, ap) in items:
                                        nc.vector.scalar_tensor_tensor(
                                            out=pts[:, o_:o_ + 128],
                                            in0=pts[:, o_:o_ + 128],
                                            scalar=1.0, in1=tri01,
                                            op0=MULT, op1=MULT)
                                pend[0] = (pts, items, y_t, pi == 0,
                                           pi == len(pairs) - 1, g, hh, qc)
                flush_y()

            if DEBUG:
                nc.sync.dma_start(out=dbg_at[0:P, :], in_=attnT[0])

            # ============= Phase 3: proj + residual + LN2 ==================
            with ExitStack() as c3:
                xo2p = c3.enter_context(tc.tile_pool(name="xo2", bufs=1))
                x_own = [xo2p.tile([P, TOWN], BF16, tag=f"xo{kt}",
                                   name=f"xo{kt}") for kt in range(KT)]
                for kt in range(KT):
                    nc.sync.dma_start(out=x_own[kt],
                                      in_=xT[kt * P:(kt + 1) * P, 0:TOWN])
                prp = c3.enter_context(tc.tile_pool(name="prp", bufs=2,
                                                    space="PSUM"))
                stp2 = c3.enter_context(tc.tile_pool(name="stp2", bufs=1,
                                                     space="PSUM"))
                sqp2 = c3.enter_context(tc.tile_pool(name="sqp2", bufs=1))
                rowp2 = c3.enter_context(tc.tile_pool(name="rowp2", bufs=1))
                bcp2 = c3.enter_context(tc.tile_pool(name="bcp2", bufs=2))

                def ln2_block(nb):
                    sl = slice(nb * 512, (nb + 1) * 512)
                    mu_b, rs_b = _ln_stats(
                        nc, [x2T[kt][:, sl] for kt in range(KT)], ones_bf,
                        eps_t, stp2, sqp2, rowp2, bcp2, f"b{nb}")
                    for kt in range(KT):
                        nc.vector.tensor_sub(out=h2T[kt][:, sl],
                                             in0=x2T[kt][:, sl], in1=mu_b)
                        nc.vector.tensor_mul(out=h2T[kt][:, sl],
                                             in0=h2T[kt][:, sl], in1=rs_b)

                # token-block-major so LN2(block0) overlaps proj(block1)
                for nbq in range(2):
                    sl = slice(nbq * 512, (nbq + 1) * 512)
                    for ft in range(KT):
                        pp = prp.tile([P, 512], F32, tag="pp",
                                      name=f"pp{nbq}_{ft}")
                        for kt in range(KT):
                            nc.tensor.matmul(
                                pp, pw[kt][:, ft * P:(ft + 1) * P],
                                attnT[kt][:, sl],
                                start=(kt == 0), stop=(kt == KT - 1))
                        nc.vector.scalar_tensor_tensor(
                            out=x2T[ft][:, sl], in0=pp,
                            scalar=projb[:, ft:ft + 1],
                            in1=x_own[ft][:, sl], op0=ADD, op1=ADD)
                    ln2_block(nbq)

        ha.close()

        # ================= Phase 4: MLP (2 chunks of 2048 dff) =============
        with ExitStack() as c4:
            w1p = c4.enter_context(tc.tile_pool(name="w1p", bufs=1))
            w2p = c4.enter_context(tc.tile_pool(name="w2p", bufs=1))
            accp = c4.enter_context(tc.tile_pool(name="accp", bufs=1))
            outp = c4.enter_context(tc.tile_pool(name="outp", bufs=3))
            f1p = c4.enter_context(tc.tile_pool(name="f1p", bufs=2, space="PSUM"))
            f2p = c4.enter_context(tc.tile_pool(name="f2p", bufs=2, space="PSUM"))
            acc = [accp.tile([P, TOWN], F32, tag=f"ac{ft}", name=f"acc{ft}")
                   for ft in range(KT)]

            for dc in range(2):
                w1 = [w1p.tile([P, 2048], BF16, tag=f"w1_{kt}",
                               name=f"w1_{dc}_{kt}", bufs=1)
                      for kt in range(KT)]
                for kt in range(KT):
                    nc.sync.dma_start(
                        out=w1[kt],
                        in_=fc1_w[kt * P:(kt + 1) * P,
                                  dc * 2048:(dc + 1) * 2048])
                w2 = [w2p.tile([P, C], BF16, tag=f"w2_{m}",
                               name=f"w2_{dc}_{m}", bufs=1)
                      for m in range(16)]
                for m in range(16):
                    nc.sync.dma_start(
                        out=w2[m],
                        in_=fc2_w[dc * 2048 + m * P:dc * 2048 + (m + 1) * P, :])
                for m in range(16):
                    f1 = f1p.tile([P, TOWN], F32, tag="f1", name=f"f1_{dc}_{m}")
                    for kt in range(KT):
                        for nbq in range(2):
                            nc.tensor.matmul(
                                f1[:, nbq * 512:(nbq + 1) * 512],
                                w1[kt][:, m * P:(m + 1) * P],
                                h2T[kt][:, nbq * 512:(nbq + 1) * 512],
                                start=(kt == 0), stop=(kt == KT - 1))
                    nc.scalar.activation(
                        out=h1[m], in_=f1, func=Relu,
                        bias=fc1b[:, dc * 16 + m:dc * 16 + m + 1])
                for ft in range(KT):
                    f2 = f2p.tile([P, TOWN], F32, tag="f2", name=f"f2_{dc}_{ft}")
                    for m in range(16):
                        for nbq in range(2):
                            nc.tensor.matmul(
                                f2[:, nbq * 512:(nbq + 1) * 512],
                                w2[m][:, ft * P:(ft + 1) * P],
                                h1[m][:, nbq * 512:(nbq + 1) * 512],
                                start=(m == 0), stop=(m == 15))
                    if dc == 0:
                        nc.vector.scalar_tensor_tensor(
                            out=acc[ft], in0=f2, scalar=fc2b[:, ft:ft + 1],
                            in1=x2T[ft], op0=ADD, op1=ADD)
                    else:
                        o = outp.tile([P, TOWN], F32, tag="o", name=f"o{ft}")
                        for hf in range(2):
                            sl2 = slice(hf * 512, (hf + 1) * 512)
                            nc.vector.tensor_add(out=o[:, sl2], in0=f2[:, sl2],
                                                 in1=acc[ft][:, sl2])
                            nc.sync.dma_start(
                                out=out[ft * P:(ft + 1) * P, sl2],
                                in_=o[:, sl2])

    nc.compile()
    return nc


_NC_CACHE = None


def _get_nc():
    global _NC_CACHE
    if _NC_CACHE is None:
        _NC_CACHE = build_nc()
    return _NC_CACHE


_CHUNKS = {0: (0, 3), 1: (1, 2)}


def _perm_chunks(j):
    cl, ch = _CHUNKS[j]
    others = [c for c in range(4) if c not in (cl, ch)]
    return [cl, ch] + others


def _make_masks(perm):
    """[9, 128, 1024] bf16 per PAIR_DEFS packing, in permuted kv order.

    Rows 0-7: additive masks (diagonal rows keep the -1e30 triangle in
    their first 128 columns per item; data rows are block-uniform).
    Row 8, cols 0:128: the 0/1 within-tile causal triangle."""
    kv_tok = np.concatenate([np.arange(c * 512, (c + 1) * 512) for c in perm])
    out = np.zeros((9, P, 1024), dtype=np.float32)
    kv = np.arange(P)[:, None]
    qq = np.arange(P)[None, :]
    out[8, :, 0:P] = (kv <= qq).astype(np.float32)
    for qc in (0, 1):
        q_tok = kv_tok[qc * 512:(qc + 1) * 512]
        for (ktA, ktB, qsA, qsB, mi) in PAIR_DEFS[qc]:
            if mi is None:
                continue
            off = 0
            for kt, qs in ((ktA, qsA), (ktB, qsB)):
                w = 512 - qs
                kvg = kv_tok[kt * P:(kt + 1) * P][:, None]
                qg = q_tok[None, qs:512]
                out[mi, :, off:off + w] = np.where(kvg <= qg, 0.0, NEG)
                off += w
    return out.astype(BF)


def _run(inputs, trace=False):
    nc = _get_nc()
    xs = {k: np.asarray(v, dtype=np.float32) for k, v in inputs.items()}
    # fold LN gains/biases into the following matmuls (host-side)
    attn_w = xs["ln1_g"][:, None] * xs["attn_w"]
    attn_b = xs["attn_b"] + xs["ln1_b"] @ xs["attn_w"]
    fc1_w = xs["ln2_g"][:, None] * xs["fc1_w"]
    fc1_b = xs["fc1_b"] + xs["ln2_b"] @ xs["fc1_w"]
    wcast = {
        "attn_w": np.ascontiguousarray(attn_w).astype(BF), "attn_b": attn_b,
        "proj_w": np.ascontiguousarray(xs["proj_w"]).astype(BF),
        "proj_b": xs["proj_b"],
        "fc1_w": np.ascontiguousarray(fc1_w).astype(BF), "fc1_b": fc1_b,
        "fc2_w": np.ascontiguousarray(xs["fc2_w"]).astype(BF),
        "fc2_b": xs["fc2_b"],
    }
    x = xs["x"]
    in_maps = []
    for c in range(8):
        b, j = divmod(c, 2)
        perm = _perm_chunks(j)
        tok = np.concatenate([np.arange(cc * 512, (cc + 1) * 512)
                              for cc in perm])
        xTh = np.ascontiguousarray(x[b].T[:, tok]).astype(BF)
        in_maps.append({"xT": xTh, "masks": _make_masks(perm), **wcast})
    res = run_bass_kernel_spmd(nc, in_maps, list(range(8)), trace=trace)
    full = np.empty((B, T, C), dtype=np.float32)
    for c in range(8):
        b, j = divmod(c, 2)
        cl, ch = _CHUNKS[j]
        o = res.results[c]["out"]            # [C, TOWN] feature-major
        full[b, cl * 512:(cl + 1) * 512] = o[:, 0:512].T
        full[b, ch * 512:(ch + 1) * 512] = o[:, 512:1024].T
    return full, res.exec_time_ns


def kernel(**inputs):
    out, _ = _run(inputs, trace=False)
    return out


# revision 27
# speedup vs baseline: 1.0157x; 1.0157x over previous
"""Trainium2 Bass kernel for a dense transformer block (nn_Block_30262339567972).

Full inputs in, full outputs out. Sharded across 8 NeuronCores with zero
collectives: core c = 2*b + j owns two 512-token chunks of batch b
(j=0 -> chunks {0,3}, j=1 -> chunks {1,2}; the pairing balances causal
attention work). The HOST PERMUTES the token axis per core to
[own_lo, own_hi, other_a, other_b] so one uniform program serves both
chunk assignments: own tokens are always columns 0-1023, and all causal
structure lives in host-computed mask data + a fixed block skip/narrow
pattern that is identical for both variants.

Everything on device is feature-major and bf16 (fp32 PSUM accumulation);
LN gains/biases are folded into the following matmul's weights on the
host, so LayerNorm on device is stats (PE ones-matmuls) + (x-mu)*rs only.
K/V/Q stay in SBUF (no DRAM spill). Attention emission is software-
pipelined (scores[i+1] lands between scores[i] and y[i] on the PE queue)
so the PE never sees a multi-microsecond idle window (keeps the HAM
clock gate at 8/8). SBUF is tight, so phases alias dead buffers:
attnT overwrites qT group-by-group, x2/h2 carve out of the dead V tiles,
and the MLP intermediate h1 reuses kT's space.
"""

from contextlib import ExitStack

import numpy as np
import ml_dtypes

import concourse.bacc as bacc
import concourse.bass as bass
import concourse.tile as tile
from concourse import mybir
from concourse.bass_utils import run_bass_kernel_spmd
import concourse.bass_utils as _bu

# NOTE: ldw-opt stays at the default (false) — walrus rejects some of this
# kernel's Ldweights under --enable-ldw-opt=true (strided V-tile weights).

F32 = mybir.dt.float32
BF16 = mybir.dt.bfloat16
P = 128
B, T, C = 4, 2048, 1024
H, D = 16, 64
DFF = 4096
TOWN = 1024
KT = C // P            # 8 feature tiles
G = H // 2             # 8 head-pair groups
NB = T // 512          # 4 token blocks of 512
EPS = 1e-5
SCALE = D ** -0.5
NEG = -1e30
BF = ml_dtypes.bfloat16
DEBUG = False

Exp = mybir.ActivationFunctionType.Exp
Relu = mybir.ActivationFunctionType.Relu
Sqrt = mybir.ActivationFunctionType.Sqrt
ADD = mybir.AluOpType.add
MULT = mybir.AluOpType.mult

# Attention pair schedule, uniform across cores (permuted kv order).
# Each entry: (ktA, ktB, qsA, qsB, mask_idx)
#   kt: kv 128-token tile in permuted order
#   qs: first query column computed for that kt (diagonal narrowing)
#   mask_idx: row in the masks input, or None for never-masked pairs
PAIR_DEFS = {
    0: [
        (0, 1, 0, 128, 0),      # own_lo diagonal
        (2, 3, 256, 384, 1),
        (8, 9, 0, 0, 2),        # other_a: data mask (all-0 or all-NEG)
        (10, 11, 0, 0, 3),
    ],
    1: [
        (0, 1, 0, 0, None),     # own_lo always fully visible from own_hi
        (2, 3, 0, 0, None),
        (4, 5, 0, 128, 4),      # own_hi diagonal
        (6, 7, 256, 384, 5),
        (8, 9, 0, 0, None),     # other_a always fully visible
        (10, 11, 0, 0, None),
        (12, 13, 0, 0, 6),      # other_b: data mask
        (14, 15, 0, 0, 7),
    ],
}


def _ln_stats(nc, src_aps, ones_bf, eps_t, stp, sqp, rowp, bcp, tag):
    """Feature-major LayerNorm stats over a 512-token block.

    src_aps: KT APs of [P, 512] bf16. Returns (mu_b, rs_b) bf16 [P, 512]
    broadcast tiles."""
    ssum = stp.tile([1, 512], F32, tag="ssum", name=f"ssum{tag}")
    ssq = stp.tile([1, 512], F32, tag="ssq", name=f"ssq{tag}")
    for kt in range(KT):
        nc.tensor.matmul(ssum, ones_bf, src_aps[kt],
                         start=(kt == 0), stop=(kt == KT - 1))
    for kt in range(KT):
        sq = sqp.tile([P, 512], BF16, tag=f"sq{kt}", name=f"sq{tag}_{kt}",
                      bufs=1)
        nc.vector.tensor_mul(out=sq, in0=src_aps[kt], in1=src_aps[kt])
        nc.tensor.matmul(ssq, ones_bf, sq,
                         start=(kt == 0), stop=(kt == KT - 1))
    mu = rowp.tile([1, 512], F32, tag="mu", name=f"mu{tag}")
    nc.vector.tensor_scalar_mul(mu, ssum, 1.0 / C)
    ms = rowp.tile([1, 512], F32, tag="ms", name=f"ms{tag}")
    nc.vector.tensor_scalar_mul(ms, ssq, 1.0 / C)
    mu2 = rowp.tile([1, 512], F32, tag="mu2", name=f"mu2{tag}")
    nc.vector.tensor_mul(out=mu2, in0=mu, in1=mu)
    nc.vector.tensor_sub(out=ms, in0=ms, in1=mu2)
    std = rowp.tile([1, 512], F32, tag="std", name=f"std{tag}")
    nc.scalar.activation(out=std, in_=ms, func=Sqrt, bias=eps_t[0:1, 0:1])
    rs = rowp.tile([1, 512], F32, tag="rs", name=f"rs{tag}")
    nc.vector.reciprocal_approx_fast(out=rs, in_=std)
    mu_bf = rowp.tile([1, 512], BF16, tag="mubf", name=f"mubf{tag}")
    nc.vector.tensor_copy(out=mu_bf, in_=mu)
    rs_bf = rowp.tile([1, 512], BF16, tag="rsbf", name=f"rsbf{tag}")
    nc.vector.tensor_copy(out=rs_bf, in_=rs)
    mu_b = bcp.tile([P, 512], BF16, tag="mub", name=f"mub{tag}")
    nc.gpsimd.partition_broadcast(mu_b, mu_bf)
    rs_b = bcp.tile([P, 512], BF16, tag="rsb", name=f"rsb{tag}")
    nc.gpsimd.partition_broadcast(rs_b, rs_bf)
    return mu_b, rs_b


def build_nc():
    nc = bacc.Bacc()
    xT = nc.declare_dram_parameter("xT", [C, T], BF16, isOutput=False)
    masks = nc.declare_dram_parameter("masks", [9, P, 1024], BF16, isOutput=False)
    attn_w = nc.declare_dram_parameter("attn_w", [C, 3 * C], BF16, isOutput=False)
    attn_b = nc.declare_dram_parameter("attn_b", [3 * C], F32, isOutput=False)
    proj_w = nc.declare_dram_parameter("proj_w", [C, C], BF16, isOutput=False)
    proj_b = nc.declare_dram_parameter("proj_b", [C], F32, isOutput=False)
    fc1_w = nc.declare_dram_parameter("fc1_w", [C, DFF], BF16, isOutput=False)
    fc1_b = nc.declare_dram_parameter("fc1_b", [DFF], F32, isOutput=False)
    fc2_w = nc.declare_dram_parameter("fc2_w", [DFF, C], BF16, isOutput=False)
    fc2_b = nc.declare_dram_parameter("fc2_b", [C], F32, isOutput=False)
    out = nc.declare_dram_parameter("out", [C, TOWN], F32, isOutput=True)
    if DEBUG:
        dbg_k = nc.declare_dram_parameter("dbg_k", [P, T], BF16, isOutput=True)
        dbg_q = nc.declare_dram_parameter("dbg_q", [P, TOWN], BF16, isOutput=True)
        dbg_v = nc.declare_dram_parameter("dbg_v", [P, 1040], BF16, isOutput=True)
        dbg_at = nc.declare_dram_parameter("dbg_at", [P, TOWN], BF16, isOutput=True)
        dbg_x2 = nc.declare_dram_parameter("dbg_x2", [P, TOWN], BF16, isOutput=True)
        dbg_h2 = nc.declare_dram_parameter("dbg_h2", [P, TOWN], BF16, isOutput=True)
        dbg_den = nc.declare_dram_parameter("dbg_den", [8, 512], F32, isOutput=True)

    with tile.TileContext(nc) as tc, ExitStack() as top:
        const = top.enter_context(tc.tile_pool(name="const", bufs=1))
        ones_bf = const.tile([P, 1], BF16, name="ones_bf")
        nc.vector.memset(ones_bf, 1.0)
        eps_t = const.tile([P, 1], F32, name="eps_t")
        nc.vector.memset(eps_t, EPS)
        abq = const.tile([P, G], F32, name="abq")
        abk = const.tile([P, G], F32, name="abk")
        nc.gpsimd.dma_start(out=abq, in_=attn_b[0:C].rearrange("(g p) -> p g", p=P))
        nc.gpsimd.dma_start(out=abk, in_=attn_b[C:2 * C].rearrange("(g p) -> p g", p=P))
        projb = const.tile([P, KT], F32, name="projb")
        nc.gpsimd.dma_start(out=projb, in_=proj_b.rearrange("(f p) -> p f", p=P))
        fc2b = const.tile([P, KT], F32, name="fc2b")
        nc.gpsimd.dma_start(out=fc2b, in_=fc2_b.rearrange("(f p) -> p f", p=P))
        fc1b = const.tile([P, DFF // P], F32, name="fc1b")
        nc.gpsimd.dma_start(out=fc1b, in_=fc1_b.rearrange("(f p) -> p f", p=P))
        bv_bc = const.tile([P, C], F32, name="bv_bc")
        abv = attn_b[2 * C:3 * C]
        nc.gpsimd.dma_start(
            out=bv_bc,
            in_=bass.AP(tensor=abv.tensor, offset=abv.offset,
                        ap=[[0, P]] + list(abv.ap[-1:])))

        # Long-lived activation state; later phases alias into dead regions.
        kvq = top.enter_context(tc.tile_pool(name="kvq", bufs=1))
        kT_t = [kvq.tile([P, T], BF16, tag=f"k{g}", name=f"kT{g}")
                for g in range(G)]
        qT_t = [kvq.tile([P, TOWN], BF16, tag=f"q{g}", name=f"qT{g}")
                for g in range(G)]
        vx = [kvq.tile([P, 1040], BF16, tag=f"v{tt}", name=f"v{tt}")
              for tt in range(T // P)]
        v_t = [t.rearrange("p (g x d) -> p g x d", g=G, x=2, d=65) for t in vx]
        for tt in range(T // P):
            nc.vector.memset(v_t[tt][:, :, :, 64:65], 1.0)
        # aliases (regions dead by the time they are written):
        attnT = qT_t                                   # written per-(g,hh,qc)
        x2T = [vx[2 * ft][:, 0:TOWN] for ft in range(KT)]
        h2T = [vx[2 * ft + 1][:, 0:TOWN] for ft in range(KT)]
        h1 = [kT_t[m // 2][:, (m % 2) * TOWN:(m % 2 + 1) * TOWN]
              for m in range(16)]

        # hT (LN1 output) stays resident through attention: K groups 2-7
        # and the V upper half are produced as interleaved "chores" inside
        # the attention phase, keeping the PE in long busy bursts so the
        # HAM clock gate ramps back to 8/8 and stays there.
        ha = ExitStack()
        hTp = ha.enter_context(tc.tile_pool(name="hTp", bufs=1))
        hT_t = [hTp.tile([P, T], BF16, tag=f"ht{kt}", name=f"hT{kt}")
                for kt in range(KT)]
        awkp = ha.enter_context(tc.tile_pool(name="awkp", bufs=1))
        awk = [awkp.tile([P, C], BF16, tag=f"awk{kt}", name=f"awk{kt}")
               for kt in range(KT)]
        awvp = ha.enter_context(tc.tile_pool(name="awvp", bufs=1))
        awv = [awvp.tile([P, C], BF16, tag=f"awv{kt}", name=f"awv{kt}")
               for kt in range(KT)]

        # ================= Phase 1: LN1 + K(g0-1)/V(lo)/Q ==================
        with ExitStack() as c1:
            # x for blocks 0/1 first so LN can start ~immediately; weights
            # stream behind them.
            for nb in (0, 1):
                for kt in range(KT):
                    eng = nc.sync if kt < 4 else nc.scalar
                    eng.dma_start(
                        out=hT_t[kt][:, nb * 512:(nb + 1) * 512],
                        in_=xT[kt * P:(kt + 1) * P, nb * 512:(nb + 1) * 512])
            awqp = c1.enter_context(tc.tile_pool(name="awqp", bufs=1))
            awq = [awqp.tile([P, C], BF16, tag=f"awq{kt}", name=f"awq{kt}")
                   for kt in range(KT)]
            for kt in range(KT):
                nc.gpsimd.dma_start(out=awq[kt],
                                    in_=attn_w[kt * P:(kt + 1) * P, 0:C])
                nc.gpsimd.dma_start(out=awk[kt],
                                    in_=attn_w[kt * P:(kt + 1) * P, C:2 * C])
                nc.gpsimd.dma_start(out=awv[kt],
                                    in_=attn_w[kt * P:(kt + 1) * P, 2 * C:3 * C])
            for nb in (2, 3):
                for kt in range(KT):
                    eng = nc.sync if kt < 4 else nc.scalar
                    eng.dma_start(
                        out=hT_t[kt][:, nb * 512:(nb + 1) * 512],
                        in_=xT[kt * P:(kt + 1) * P, nb * 512:(nb + 1) * 512])

            sqp = c1.enter_context(tc.tile_pool(name="sqp", bufs=1))
            stp = c1.enter_context(tc.tile_pool(name="stp", bufs=2, space="PSUM"))
            rowp = c1.enter_context(tc.tile_pool(name="rowp", bufs=2))
            bcp = c1.enter_context(tc.tile_pool(name="bcp", bufs=2))
            mmp = c1.enter_context(tc.tile_pool(name="mmp", bufs=2, space="PSUM"))

            # depth-2 stats pipeline: stats(nb+1)'s PE matmuls are emitted
            # before block nb's apply/KQV so the PE never waits on the
            # serial DVE stats chain.
            def _stats(nb_):
                xb_ = [hT_t[kt][:, nb_ * 512:(nb_ + 1) * 512]
                       for kt in range(KT)]
                return _ln_stats(nc, xb_, ones_bf, eps_t, stp, sqp,
                                 rowp, bcp, f"a{nb_}")

            stats = {0: _stats(0)}
            for nb in range(NB):
                if nb + 1 < NB:
                    stats[nb + 1] = _stats(nb + 1)
                sl = slice(nb * 512, (nb + 1) * 512)
                mu_b, rs_b = stats.pop(nb)
                # in-place apply: hT <- (x - mu) * rs
                for kt in range(KT):
                    nc.vector.tensor_sub(out=hT_t[kt][:, sl],
                                         in0=hT_t[kt][:, sl], in1=mu_b)
                    nc.vector.tensor_mul(out=hT_t[kt][:, sl],
                                         in0=hT_t[kt][:, sl], in1=rs_b)
                hT = [hT_t[kt][:, sl] for kt in range(KT)]

                for g in (0, 1):
                    kps = mmp.tile([P, 512], F32, tag="kq", name=f"kps{nb}_{g}")
                    for kt in range(KT):
                        nc.tensor.matmul(
                            kps, awk[kt][:, g * P:(g + 1) * P], hT[kt],
                            start=(kt == 0), stop=(kt == KT - 1))
                    nc.vector.tensor_scalar_add(
                        out=kT_t[g][:, sl], in0=kps, scalar1=abk[:, g:g + 1])
                if nb < 2:
                    for g in range(G):
                        qps = mmp.tile([P, 512], F32, tag="kq",
                                       name=f"qps{nb}_{g}")
                        for kt in range(KT):
                            nc.tensor.matmul(
                                qps, awq[kt][:, g * P:(g + 1) * P], hT[kt],
                                start=(kt == 0), stop=(kt == KT - 1))
                        nc.vector.tensor_scalar_add(
                            out=qT_t[g][:, sl], in0=qps,
                            scalar1=abq[:, g:g + 1])

                # --- V lower feature half (groups 0-3) ---
                for t4 in range(4):
                    tt = nb * 4 + t4
                    vps = mmp.tile([P, 512], F32, tag="v0", name=f"vps{tt}")
                    for kt in range(KT):
                        nc.tensor.matmul(
                            vps, hT[kt][:, t4 * P:(t4 + 1) * P],
                            awv[kt][:, 0:512],
                            start=(kt == 0), stop=(kt == KT - 1))
                    nc.vector.tensor_add(
                        out=v_t[tt][:, 0:4, :, 0:64],
                        in0=vps.rearrange("p (g x d) -> p g x d", x=2, d=64),
                        in1=bv_bc[:, 0:512].rearrange("p (g x d) -> p g x d",
                                                      x=2, d=64))

        if DEBUG:
            nc.sync.dma_start(out=dbg_k[0:P, :], in_=kT_t[0])
            nc.sync.dma_start(out=dbg_q[0:P, :], in_=qT_t[0])
            nc.sync.dma_start(out=dbg_v[0:P, :], in_=vx[0])

        # ================= Phase 2: attention ==============================
        with ExitStack() as cb:
            pwp = cb.enter_context(tc.tile_pool(name="pwp", bufs=1))
            pw = [pwp.tile([P, C], BF16, tag=f"pw{kt}", name=f"pw{kt}")
                  for kt in range(KT)]
            for kt in range(KT):
                nc.sync.dma_start(out=pw[kt], in_=proj_w[kt * P:(kt + 1) * P, :])

            with ExitStack() as c2:
                mkp = c2.enter_context(tc.tile_pool(name="mkp", bufs=1))
                mk = [mkp.tile([P, 1024], BF16, tag=f"mk{i}", name=f"mk{i}")
                      for i in range(9)]
                for i in range(9):
                    nc.sync.dma_start(out=mk[i], in_=masks[i])
                tri01 = mk[8][:, 0:128]
                # data masks are block-uniform: fold scale*mask into the exp
                # bias (per-partition column). Diagonal masks act post-exp as
                # a 0/1 triangle multiply on gpsimd (SBUF-only engine).
                mbias = []
                for i in range(8):
                    mb = mkp.tile([P, 1], F32, tag=f"mb{i}", name=f"mb{i}")
                    nc.vector.tensor_scalar_mul(mb, mk[i][:, 0:1], SCALE)
                    mbias.append(mb)
                scp = c2.enter_context(tc.tile_pool(name="scp", bufs=2,
                                                    space="PSUM"))
                yp = c2.enter_context(tc.tile_pool(name="yp", bufs=2,
                                                   space="PSUM"))
                chm = c2.enter_context(tc.tile_pool(name="chm", bufs=2,
                                                    space="PSUM"))

                # deferred K (groups 2-7) and V upper half, emitted as
                # chores between attention pairs: each chore is an 8-matmul
                # PSUM chain (a sustained PE burst for the HAM clock gate).
                def k_chore(g, nb):
                    def run():
                        sl = slice(nb * 512, (nb + 1) * 512)
                        kps = chm.tile([P, 512], F32, tag="ch",
                                       name=f"dkps{g}_{nb}")
                        for kt in range(KT):
                            nc.tensor.matmul(
                                kps, awk[kt][:, g * P:(g + 1) * P],
                                hT_t[kt][:, sl],
                                start=(kt == 0), stop=(kt == KT - 1))
                        nc.vector.tensor_scalar_add(
                            out=kT_t[g][:, sl], in0=kps,
                            scalar1=abk[:, g:g + 1])
                    return run

                def v_chore(tt):
                    def run():
                        vps = chm.tile([P, 512], F32, tag="ch",
                                       name=f"dvps{tt}")
                        for kt in range(KT):
                            nc.tensor.matmul(
                                vps, hT_t[kt][:, tt * P:(tt + 1) * P],
                                awv[kt][:, 512:1024],
                                start=(kt == 0), stop=(kt == KT - 1))
                        nc.vector.tensor_add(
                            out=v_t[tt][:, 4:8, :, 0:64],
                            in0=vps.rearrange("p (g x d) -> p g x d",
                                              x=2, d=64),
                            in1=bv_bc[:, 512:1024].rearrange(
                                "p (g x d) -> p g x d", x=2, d=64))
                    return run

                chores = []
                for g_ in (2, 3):
                    chores += [k_chore(g_, nb_) for nb_ in range(NB)]
                chores += [v_chore(tt_) for tt_ in range(T // P)]
                for g_ in (4, 5, 6, 7):
                    chores += [k_chore(g_, nb_) for nb_ in range(NB)]
                chores.reverse()   # pop() from the front
                ptp = c2.enter_context(tc.tile_pool(name="ptp", bufs=3))
                rcp = c2.enter_context(tc.tile_pool(name="rcp", bufs=2))
                rbp = c2.enter_context(tc.tile_pool(name="rbp", bufs=2))

                # software-pipelined emission: scores(i+1) lands on the PE
                # queue between scores(i) and y(i) so the PE never waits a
                # full mask+exp latency. Normalize is emitted right after a
                # y-group's last matmul; the pipeline flows across qc/hh/g.
                pend = [None]

                def norm(y_t, g, hh, qc):
                    hsl = slice(64 * hh, 64 * (hh + 1))
                    den = rcp.tile([1, 512], F32, tag="den",
                                   name=f"den{g}_{hh}_{qc}")
                    nc.vector.tensor_copy(out=den, in_=y_t[64:65, :])
                    rc = rcp.tile([1, 512], F32, tag="rc",
                                  name=f"rc{g}_{hh}_{qc}")
                    nc.vector.reciprocal_approx_fast(out=rc, in_=den)
                    if DEBUG and g == 0 and hh == 0:
                        nc.sync.dma_start(out=dbg_den[2 * qc:2 * qc + 1, :],
                                          in_=den)
                        nc.sync.dma_start(out=dbg_den[2 * qc + 1:2 * qc + 2, :],
                                          in_=rc)
                    rb = rbp.tile([64, 512], F32, tag="rb",
                                  name=f"rb{g}_{hh}_{qc}")
                    nc.gpsimd.partition_broadcast(rb, rc)
                    nc.vector.tensor_mul(
                        out=attnT[g][hsl, qc * 512:(qc + 1) * 512],
                        in0=y_t[0:64, :], in1=rb)

                def flush_y():
                    if pend[0] is None:
                        return
                    pts, items, y_t, first, last, g_, hh_, qc_ = pend[0]
                    for idx, (kt, off, qs, ap) in enumerate(items):
                        nc.tensor.matmul(
                            y_t[:, qs:qs + ap], v_t[kt][:, g_, hh_, :],
                            pts[:, off:off + ap],
                            start=(first and idx == 0),
                            stop=(last and idx == len(items) - 1))
                    if last:
                        norm(y_t, g_, hh_, qc_)
                    pend[0] = None

                for g in range(G):
                    for hh in range(2):
                        hsl = slice(64 * hh, 64 * (hh + 1))
                        for qc in (0, 1):
                            pairs = PAIR_DEFS[qc]
                            y_t = yp.tile([65, 512], F32, tag="y",
                                          name=f"y{g}_{hh}_{qc}")
                            for pi, (ktA, ktB, qsA, qsB, mi) in enumerate(pairs):
                                items = []
                                off = 0
                                for kt, qs in ((ktA, qsA), (ktB, qsB)):
                                    items.append((kt, off, qs, 512 - qs))
                                    off += 512 - qs
                                w = off
                                scs = scp.tile([P, 1024], F32, tag="sc",
                                               name=f"sc{g}_{hh}_{qc}_{pi}")
                                for (kt, o_, qs, ap) in items:
                                    nc.tensor.matmul(
                                        scs[:, o_:o_ + ap],
                                        kT_t[g][hsl, kt * P:(kt + 1) * P],
                                        qT_t[g][hsl,
                                                qc * 512 + qs:(qc + 1) * 512],
                                        start=True, stop=True,
                                        tile_position=(64 * hh, 0))
                                flush_y()
                                if chores:
                                    chores.pop()()
                                if chores:
                                    chores.pop()()
                                diag = mi is not None and (qsA or qsB)
                                pts = ptp.tile([P, 1024], BF16, tag="pt",
                                               name=f"pt{g}_{hh}_{qc}_{pi}")
                                nc.scalar.activation(
                                    out=pts[:, 0:w], in_=scs[:, 0:w],
                                    func=Exp, scale=SCALE,
                                    bias=(mbias[mi][:, 0:1]
                                          if (mi is not None and not diag)
                                          else 0.0))
                                if diag:
                                    for (kt, o_, qs, ap) in items:
                                        nc.vector.scalar_tensor_tensor(
                                            out=pts[:, o_:o_ + 128],
                                            in0=pts[:, o_:o_ + 128],
                                            scalar=1.0, in1=tri01,
                                            op0=MULT, op1=MULT)
                                pend[0] = (pts, items, y_t, pi == 0,
                                           pi == len(pairs) - 1, g, hh, qc)
                flush_y()

            if DEBUG:
                nc.sync.dma_start(out=dbg_at[0:P, :], in_=attnT[0])

            # ============= Phase 3: proj + residual + LN2 ==================
            with ExitStack() as c3:
                xo2p = c3.enter_context(tc.tile_pool(name="xo2", bufs=1))
                x_own = [xo2p.tile([P, TOWN], BF16, tag=f"xo{kt}",
                                   name=f"xo{kt}") for kt in range(KT)]
                for kt in range(KT):
                    nc.sync.dma_start(out=x_own[kt],
                                      in_=xT[kt * P:(kt + 1) * P, 0:TOWN])
                prp = c3.enter_context(tc.tile_pool(name="prp", bufs=2,
                                                    space="PSUM"))
                stp2 = c3.enter_context(tc.tile_pool(name="stp2", bufs=1,
                                                     space="PSUM"))
                sqp2 = c3.enter_context(tc.tile_pool(name="sqp2", bufs=1))
                rowp2 = c3.enter_context(tc.tile_pool(name="rowp2", bufs=1))
                bcp2 = c3.enter_context(tc.tile_pool(name="bcp2", bufs=2))

                def ln2_block(nb):
                    sl = slice(nb * 512, (nb + 1) * 512)
                    mu_b, rs_b = _ln_stats(
                        nc, [x2T[kt][:, sl] for kt in range(KT)], ones_bf,
                        eps_t, stp2, sqp2, rowp2, bcp2, f"b{nb}")
                    for kt in range(KT):
                        nc.vector.tensor_sub(out=h2T[kt][:, sl],
                                             in0=x2T[kt][:, sl], in1=mu_b)
                        nc.vector.tensor_mul(out=h2T[kt][:, sl],
                                             in0=h2T[kt][:, sl], in1=rs_b)

                # token-block-major so LN2(block0) overlaps proj(block1)
                for nbq in range(2):
                    sl = slice(nbq * 512, (nbq + 1) * 512)
                    for ft in range(KT):
                        pp = prp.tile([P, 512], F32, tag="pp",
                                      name=f"pp{nbq}_{ft}")
                        for kt in range(KT):
                            nc.tensor.matmul(
                                pp, pw[kt][:, ft * P:(ft + 1) * P],
                                attnT[kt][:, sl],
                                start=(kt == 0), stop=(kt == KT - 1))
                        nc.vector.scalar_tensor_tensor(
                            out=x2T[ft][:, sl], in0=pp,
                            scalar=projb[:, ft:ft + 1],
                            in1=x_own[ft][:, sl], op0=ADD, op1=ADD)
                    ln2_block(nbq)

        ha.close()

        # ================= Phase 4: MLP (2 chunks of 2048 dff) =============
        with ExitStack() as c4:
            w1p = c4.enter_context(tc.tile_pool(name="w1p", bufs=1))
            w2p = c4.enter_context(tc.tile_pool(name="w2p", bufs=1))
            accp = c4.enter_context(tc.tile_pool(name="accp", bufs=1))
            outp = c4.enter_context(tc.tile_pool(name="outp", bufs=3))
            f1p = c4.enter_context(tc.tile_pool(name="f1p", bufs=2, space="PSUM"))
            f2p = c4.enter_context(tc.tile_pool(name="f2p", bufs=2, space="PSUM"))
            acc = [accp.tile([P, TOWN], F32, tag=f"ac{ft}", name=f"acc{ft}")
                   for ft in range(KT)]

            for dc in range(2):
                w1 = [w1p.tile([P, 2048], BF16, tag=f"w1_{kt}",
                               name=f"w1_{dc}_{kt}", bufs=1)
                      for kt in range(KT)]
                for kt in range(KT):
                    nc.sync.dma_start(
                        out=w1[kt],
                        in_=fc1_w[kt * P:(kt + 1) * P,
                                  dc * 2048:(dc + 1) * 2048])
                w2 = [w2p.tile([P, C], BF16, tag=f"w2_{m}",
                               name=f"w2_{dc}_{m}", bufs=1)
                      for m in range(16)]
                for m in range(16):
                    nc.sync.dma_start(
                        out=w2[m],
                        in_=fc2_w[dc * 2048 + m * P:dc * 2048 + (m + 1) * P, :])
                for m in range(16):
                    f1 = f1p.tile([P, TOWN], F32, tag="f1", name=f"f1_{dc}_{m}")
                    for kt in range(KT):
                        for nbq in range(2):
                            nc.tensor.matmul(
                                f1[:, nbq * 512:(nbq + 1) * 512],
                                w1[kt][:, m * P:(m + 1) * P],
                                h2T[kt][:, nbq * 512:(nbq + 1) * 512],
                                start=(kt == 0), stop=(kt == KT - 1))
                    nc.scalar.activation(
                        out=h1[m], in_=f1, func=Relu,
                        bias=fc1b[:, dc * 16 + m:dc * 16 + m + 1])
                for ft in range(KT):
                    f2 = f2p.tile([P, TOWN], F32, tag="f2", name=f"f2_{dc}_{ft}")
                    for m in range(16):
                        for nbq in range(2):
                            nc.tensor.matmul(
                                f2[:, nbq * 512:(nbq + 1) * 512],
                                w2[m][:, ft * P:(ft + 1) * P],
                                h1[m][:, nbq * 512:(nbq + 1) * 512],
                                start=(m == 0), stop=(m == 15))
                    if dc == 0:
                        nc.vector.scalar_tensor_tensor(
                            out=acc[ft], in0=f2, scalar=fc2b[:, ft:ft + 1],
                            in1=x2T[ft], op0=ADD, op1=ADD)
                    else:
                        o = outp.tile([P, TOWN], F32, tag="o", name=f"o{ft}")
                        for hf in range(2):
                            sl2 = slice(hf * 512, (hf + 1) * 512)
                            nc.vector.tensor_add(out=o[:, sl2], in0=f2[:, sl2],
                                                 in1=acc[ft][:, sl2])
                            nc.sync.dma_start(
                                out=out[ft * P:(ft + 1) * P, sl2],
                                in_=o[:, sl2])

    nc.compile()
    return nc


_NC_CACHE = None


def _get_nc():
    global _NC_CACHE
    if _NC_CACHE is None:
        _NC_CACHE = build_nc()
    return _NC_CACHE


_CHUNKS = {0: (0, 3), 1: (1, 2)}


def _perm_chunks(j):
    cl, ch = _CHUNKS[j]
    others = [c for c in range(4) if c not in (cl, ch)]
    return [cl, ch] + others


def _make_masks(perm):
    """[9, 128, 1024] bf16 per PAIR_DEFS packing, in permuted kv order.

    Rows 0-7: additive masks (diagonal rows keep the -1e30 triangle in
    their first 128 columns per item; data rows are block-uniform).
    Row 8, cols 0:128: the 0/1 within-tile causal triangle."""
    kv_tok = np.concatenate([np.arange(c * 512, (c + 1) * 512) for c in perm])
    out = np.zeros((9, P, 1024), dtype=np.float32)
    kv = np.arange(P)[:, None]
    qq = np.arange(P)[None, :]
    out[8, :, 0:P] = (kv <= qq).astype(np.float32)
    for qc in (0, 1):
        q_tok = kv_tok[qc * 512:(qc + 1) * 512]
        for (ktA, ktB, qsA, qsB, mi) in PAIR_DEFS[qc]:
            if mi is None:
                continue
            off = 0
            for kt, qs in ((ktA, qsA), (ktB, qsB)):
                w = 512 - qs
                kvg = kv_tok[kt * P:(kt + 1) * P][:, None]
                qg = q_tok[None, qs:512]
                out[mi, :, off:off + w] = np.where(kvg <= qg, 0.0, NEG)
                off += w
    return out.astype(BF)


def _run(inputs, trace=False):
    nc = _get_nc()
    xs = {k: np.asarray(v, dtype=np.float32) for k, v in inputs.items()}
    # fold LN gains/biases into the following matmuls (host-side)
    attn_w = xs["ln1_g"][:, None] * xs["attn_w"]
    attn_b = xs["attn_b"] + xs["ln1_b"] @ xs["attn_w"]
    fc1_w = xs["ln2_g"][:, None] * xs["fc1_w"]
    fc1_b = xs["fc1_b"] + xs["ln2_b"] @ xs["fc1_w"]
    wcast = {
        "attn_w": np.ascontiguousarray(attn_w).astype(BF), "attn_b": attn_b,
        "proj_w": np.ascontiguousarray(xs["proj_w"]).astype(BF),
        "proj_b": xs["proj_b"],
        "fc1_w": np.ascontiguousarray(fc1_w).astype(BF), "fc1_b": fc1_b,
        "fc2_w": np.ascontiguousarray(xs["fc2_w"]).astype(BF),
        "fc2_b": xs["fc2_b"],
    }
    x = xs["x"]
    in_maps = []
    for c in range(8):
        b, j = divmod(c, 2)
        perm = _perm_chunks(j)
        tok = np.concatenate([np.arange(cc * 512, (cc + 1) * 512)
                              for cc in perm])
        xTh = np.ascontiguousarray(x[b].T[:, tok]).astype(BF)
        in_maps.append({"xT": xTh, "masks": _make_masks(perm), **wcast})
    res = run_bass_kernel_spmd(nc, in_maps, list(range(8)), trace=trace)
    full = np.empty((B, T, C), dtype=np.float32)
    for c in range(8):
        b, j = divmod(c, 2)
        cl, ch = _CHUNKS[j]
        o = res.results[c]["out"]            # [C, TOWN] feature-major
        full[b, cl * 512:(cl + 1) * 512] = o[:, 0:512].T
        full[b, ch * 512:(ch + 1) * 512] = o[:, 512:1024].T
    return full, res.exec_time_ns


def kernel(**inputs):
    out, _ = _run(inputs, trace=False)
    return out
